# revision 11
# baseline (speedup 1.0000x reference)
"""Trainium2 Bass kernel for LoRAIPAttnProcessor (reduces to plain MHA).

Math (LORA_SCALE=0, IP_SCALE=0, b_out=0 contributions handled host-side):
  q = x @ Wq.T * scale ; k = x @ Wk.T ; v = x @ Wv.T
  P = softmax(q k^T) per head (8 heads, head_dim 160)
  out = (P v) @ Wout.T + b_out

Sharding: data-parallel over batch. 16 batches -> 8 cores x 2 batches.

Device layout strategy (zero on-device transposes):
  - host supplies xT [1280, 2048] (features on partitions) in bf16
  - host supplies Wq.T/Wk.T with *columns permuted* so each head's first 128
    output dims form full 128-partition tiles 0..7 and the 8x32 tails pack
    into tiles 8,9.  Wout.T gets the matching *row* permutation.
  - scores are computed transposed: ST[j,i] = k q^T  (keys on partitions), so
    softmax exp is a pure elementwise ACT op and P[j,i] feeds the PV matmul
    directly as the moving operand: OT[d,i] = v[j,d].T @ P[j,i].
  - a ones-column appended to v gives the softmax denominator as an extra
    output row of OT; normalization happens on DVE via a PE rank-1-broadcast
    reciprocal after otm/ott are evicted to SBUF.
  - out-projection consumes OT tiles as stationary -> final lands [token, ch].

Perf notes (HW-measured; the CoreSim cost model misses most of these):
  - matmuls whose contraction uses <128 partitions stream at ~half rate, so
    the 32-row k-tails are stored zero-padded to 128 partitions at their
    packed row offset (ktp tiles); the packed q-tail tile then works as the
    moving operand unchanged (garbage rows hit zero weights).
  - every matmul pays its own ldweights (no dedup, walrus ldw-opt off), so
    v-projection uses 512-wide psum chunks (30 matmuls/j-tile instead of 80).
  - pv(h, j) lags one scores step globally (across head boundaries) and
    otm/ott psums are evicted to SBUF immediately, so the PE never stalls on
    the last exp of a head while normalization runs off-path on DVE.
  - x/wq DMAs are issue-interleaved and wpool holds 20 tiles so weight
    streams (wk/wv/wo) prefetch behind compute.
"""

import numpy as np
import ml_dtypes
from contextlib import ExitStack

import concourse.bass as bass
import concourse.bacc as bacc
import concourse.mybir as mybir
import concourse.tile as tile
from concourse.bass_utils import run_bass_kernel_spmd

HS = 1280
HEADS = 8
D = HS // HEADS           # 160
B = 16
S = 1024
NCORES = 8
BPC = B // NCORES         # 2 batches per core
TOK = BPC * S             # 2048 tokens per core
SCALE = D ** -0.5
CT = HS // 128            # 10 feature tiles
IC = 512                  # i (query) chunk for psum
JT = S // 128             # 8 key tiles per batch
MT = S // 128             # 8 token tiles per batch

BF16 = mybir.dt.bfloat16
F32 = mybir.dt.float32
EXP = mybir.ActivationFunctionType.Exp

VW = D + 1                # 161: per-head v width incl ones column


def _perm():
    """Output-feature permutation: head mains to tiles 0..7, tails packed 8..9."""
    p = []
    for h in range(HEADS):
        p.extend(range(D * h, D * h + 128))
    for h in range(HEADS):
        p.extend(range(D * h + 128, D * h + D))
    return np.array(p, dtype=np.int64)


def _body(ctx, tc, xT_d, wq_d, wk_d, wv_d, wo_d, out_d):
    nc = tc.nc

    wpool = ctx.enter_context(tc.tile_pool(name="w", bufs=20))
    xpool = ctx.enter_context(tc.tile_pool(name="x", bufs=CT))
    qpool = ctx.enter_context(tc.tile_pool(name="q", bufs=CT))
    kpool = ctx.enter_context(tc.tile_pool(name="k", bufs=CT - 2))
    tpool = ctx.enter_context(tc.tile_pool(name="ktp", bufs=HEADS))
    vpool = ctx.enter_context(tc.tile_pool(name="v", bufs=JT))
    opool = ctx.enter_context(tc.tile_pool(name="ot", bufs=CT))
    ppool = ctx.enter_context(tc.tile_pool(name="p", bufs=3))
    rpool = ctx.enter_context(tc.tile_pool(name="recip", bufs=2))
    bpool = ctx.enter_context(tc.tile_pool(name="bcast", bufs=2))
    epool = ctx.enter_context(tc.tile_pool(name="evict", bufs=2))
    pr_ps = ctx.enter_context(tc.tile_pool(name="pr_ps", bufs=2, space="PSUM"))
    st_ps = ctx.enter_context(tc.tile_pool(name="st_ps", bufs=2, space="PSUM"))
    om_ps = ctx.enter_context(tc.tile_pool(name="om_ps", bufs=2, space="PSUM"))
    ot_ps = ctx.enter_context(tc.tile_pool(name="ot_ps", bufs=2, space="PSUM"))

    ones = rpool.tile([1, 128], F32, tag="ones", name="ones")
    nc.vector.memset(ones[:], 1.0)

    # k-tail tiles, zero-padded to full 128 contraction partitions: head h's
    # 32 tail rows live at their packed offset 32*(h%4); all other rows stay
    # zero so the packed q-tail tile works as the moving operand unchanged.
    # Allocated once; zeros persist across batches (only data rows rewritten).
    ktp = [tpool.tile([128, S], BF16, tag="ktp", name="ktp") for _ in range(HEADS)]
    for h in range(HEADS):
        nc.vector.memset(ktp[h][:], 0.0)

    for b in range(BPC):
        # ---- load this batch's xT, interleaved with wq so the first q-proj
        # accumulation chain can start as soon as tile pair 0 lands ----
        xb = []
        wq_t, wk_t, wv_t = [], [], []
        for c in range(CT):
            t = xpool.tile([128, S], BF16, tag="xb", name="xb")
            nc.sync.dma_start(out=t[:], in_=xT_d[c * 128:(c + 1) * 128, b * S:(b + 1) * S])
            xb.append(t)
            t = wpool.tile([128, HS], BF16, tag="w", name="w")
            nc.sync.dma_start(out=t[:], in_=wq_d[c * 128:(c + 1) * 128, :])
            wq_t.append(t)
        for c in range(CT):
            t = wpool.tile([128, HS], BF16, tag="w", name="w")
            nc.sync.dma_start(out=t[:], in_=wk_d[c * 128:(c + 1) * 128, :])
            wk_t.append(t)
        for c in range(CT):
            t = wpool.tile([128, HS], BF16, tag="w", name="w")
            nc.sync.dma_start(out=t[:], in_=wv_d[c * 128:(c + 1) * 128, :])
            wv_t.append(t)

        # ---- q/k projections: dst[m][dout 128, i] = W.T[c, dout_m] . xT[c, i] ----
        qT, kT = [], []
        for wt, dst, dtag in ((wq_t, qT, "qT"), (wk_t, kT, "kT")):
            for m in range(CT):
                is_k_tail = dst is kT and m >= 8
                if is_k_tail:
                    dtile = None
                else:
                    dtile = (qpool if dst is qT else kpool).tile(
                        [128, S], BF16, tag=dtag, name=dtag)
                dst.append(dtile)
                for ic in range(S // IC):
                    ps = pr_ps.tile([128, IC], F32, tag="pr", name="pr")
                    for c in range(CT):
                        nc.tensor.matmul(
                            ps[:],
                            wt[c][:, m * 128:(m + 1) * 128],
                            xb[c][:, ic * IC:(ic + 1) * IC],
                            start=(c == 0), stop=(c == CT - 1),
                        )
                    if is_k_tail:
                        # scatter the four heads' tail rows into their padded tiles
                        for hh in range(4):
                            h = (m - 8) * 4 + hh
                            r = 32 * hh
                            nc.vector.tensor_copy(
                                ktp[h][r:r + 32, ic * IC:(ic + 1) * IC],
                                ps[r:r + 32, :])
                    else:
                        nc.vector.tensor_copy(dtile[:, ic * IC:(ic + 1) * IC], ps[:])

        # ---- v projection: v'[j][tok 128, h*161 + d] (+ ones col per head)
        # 512-wide psum chunks (3.2 heads each); evictions split on head
        # boundaries so the strided 161-per-head layout lands right ----
        vp = []
        for j in range(JT):
            vt = vpool.tile([128, HEADS * VW], BF16, tag="vp", name="vp")
            vp.append(vt)
            for n0, nw in ((0, 512), (512, 512), (1024, 256)):
                ps = pr_ps.tile([128, nw], F32, tag="pr", name="pr")
                for c in range(CT):
                    nc.tensor.matmul(
                        ps[:],
                        xb[c][:, j * 128:(j + 1) * 128],
                        wv_t[c][:, n0:n0 + nw],
                        start=(c == 0), stop=(c == CT - 1),
                    )
                # scatter chunk cols [n0, n0+nw) into per-head 161-strided vt
                col = n0
                while col < n0 + nw:
                    h, d0 = divmod(col, D)
                    seg = min(D - d0, n0 + nw - col)
                    nc.vector.tensor_copy(
                        vt[:, h * VW + d0:h * VW + d0 + seg],
                        ps[:, col - n0:col - n0 + seg])
                    col += seg
            for h in range(HEADS):
                nc.vector.memset(vt[:, h * VW + D:(h + 1) * VW], 1.0)

        # ---- attention, software-pipelined across heads ----
        # pv(h, j) lags one scores step behind globally, so head h's final pv
        # is emitted after head h+1's first scores block and the PE stream
        # never waits on the exp of the last j.  otm/ott psums are evicted to
        # SBUF immediately after the final pv so the next head's accumulation
        # can claim the banks while normalization runs on DVE off-path.
        OT = [opool.tile([128, S], BF16, tag="ot", name="ot") for _ in range(CT)]
        hstate = {}

        def emit_pv(h, j):
            stt = hstate[h]
            for ic in range(2):
                nc.tensor.matmul(
                    stt["otm"][ic][:],
                    vp[j][:, h * VW:h * VW + 128],
                    stt["pj"][j][:, ic * IC:(ic + 1) * IC],
                    start=(j == 0), stop=(j == JT - 1),
                )
                nc.tensor.matmul(
                    stt["ott"][ic][:],
                    vp[j][:, h * VW + 128:(h + 1) * VW],
                    stt["pj"][j][:, ic * IC:(ic + 1) * IC],
                    start=(j == 0), stop=(j == JT - 1),
                )

        def finish_head(h):
            stt = hstate.pop(h)
            g = 8 + h // 4          # tail tile index
            r = 32 * (h % 4)        # tail row offset
            for ic in range(2):
                ou = bpool.tile([128, IC], F32, tag="ou", name="ou")
                ol = rpool.tile([33, IC], F32, tag="ol", name="ol")
                nc.vector.tensor_copy(ou[:], stt["otm"][ic][:])
                nc.vector.tensor_copy(ol[:], stt["ott"][ic][:])
                rc = rpool.tile([1, IC], F32, tag="rc", name="rc")
                nc.vector.reciprocal(rc[:], ol[32:33, :])
                # rank-1 broadcast on PE: ones.T @ rc -> [128, IC] psum
                bc_ps = pr_ps.tile([128, IC], F32, tag="pr", name="pr")
                nc.tensor.matmul(bc_ps[:], ones[:], rc[:], start=True, stop=True)
                sl = slice(ic * IC, (ic + 1) * IC)
                nc.vector.tensor_mul(OT[h][:, sl], ou[:], bc_ps[:])
                nc.vector.tensor_mul(OT[g][r:r + 32, sl], ol[0:32, :], bc_ps[0:32, :])

        pend = None
        for h in range(HEADS):
            km, kt = kT[h], ktp[h]
            qm, qt = qT[h], qT[8 + h // 4]
            for j in range(JT):
                pj_t = ppool.tile([128, S], BF16, tag="pj", name="pj")
                for ic in range(2):
                    st = st_ps.tile([128, IC], F32, tag="st", name="st")
                    nc.tensor.matmul(
                        st[:],
                        km[:, j * 128:(j + 1) * 128],
                        qm[:, ic * IC:(ic + 1) * IC],
                        start=True, stop=False,
                    )
                    # tail contraction at full 128 partitions: kt rows outside
                    # this head's 32 tail rows are zero, so qt's other heads'
                    # rows contribute nothing.
                    nc.tensor.matmul(
                        st[:],
                        kt[:, j * 128:(j + 1) * 128],
                        qt[:, ic * IC:(ic + 1) * IC],
                        start=False, stop=True,
                    )
                    nc.scalar.activation(pj_t[:, ic * IC:(ic + 1) * IC], st[:], EXP)
                if pend is not None:
                    emit_pv(*pend)
                    if pend[1] == JT - 1:
                        finish_head(pend[0])
                if j == 0:
                    # allocate after the previous head's final pv + eviction
                    # are emitted, so the psum slot handoff is ordered
                    hstate[h] = {
                        "otm": [om_ps.tile([128, IC], F32, tag="om", name="om")
                                for _ in range(2)],
                        "ott": [ot_ps.tile([33, IC], F32, tag="otl", name="otl")
                                for _ in range(2)],
                        "pj": [None] * JT,
                    }
                hstate[h]["pj"][j] = pj_t
                pend = (h, j)
        emit_pv(*pend)
        finish_head(pend[0])

        # ---- out projection: out[i, cout] = OT[d, i].T . Wout.T[d, cout] ----
        wo_t = []
        for c in range(CT):
            t = wpool.tile([128, HS], BF16, tag="w", name="w")
            nc.sync.dma_start(out=t[:], in_=wo_d[c * 128:(c + 1) * 128, :])
            wo_t.append(t)
        for it in range(MT):
            for n0, nw in ((0, 512), (512, 512), (1024, 256)):
                ps = pr_ps.tile([128, nw], F32, tag="pr", name="pr")
                for c in range(CT):
                    nc.tensor.matmul(
                        ps[:],
                        OT[c][:, it * 128:(it + 1) * 128],
                        wo_t[c][:, n0:n0 + nw],
                        start=(c == 0), stop=(c == CT - 1),
                    )
                ev = epool.tile([128, nw], F32, tag="ev", name="ev")
                nc.vector.tensor_copy(ev[:], ps[:])
                nc.sync.dma_start(
                    out=out_d[b * S + it * 128: b * S + (it + 1) * 128, n0:n0 + nw],
                    in_=ev[:],
                )


_CACHE = {}


def _build(reps=1):
    key = ("nc", reps)
    if key in _CACHE:
        return _CACHE[key]
    nc = bacc.Bacc(None)
    xT_d = nc.declare_dram_parameter("xT", [HS, TOK], BF16, isOutput=False)
    wq_d = nc.declare_dram_parameter("wq", [HS, HS], BF16, isOutput=False)
    wk_d = nc.declare_dram_parameter("wk", [HS, HS], BF16, isOutput=False)
    wv_d = nc.declare_dram_parameter("wv", [HS, HS], BF16, isOutput=False)
    wo_d = nc.declare_dram_parameter("wo", [HS, HS], BF16, isOutput=False)
    out_d = nc.declare_dram_parameter("out", [TOK, HS], F32, isOutput=True)
    with tile.TileContext(nc) as tc:
        for _ in range(reps):
            with ExitStack() as ctx:
                _body(ctx, tc, xT_d[:], wq_d[:], wk_d[:], wv_d[:], wo_d[:], out_d[:])
    nc.compile()
    _CACHE[key] = nc
    return nc


def _prep_in_maps(inputs):
    hs = np.asarray(inputs["hidden_states"], dtype=np.float32)
    perm = _perm()
    bf = ml_dtypes.bfloat16
    wq = np.ascontiguousarray((np.asarray(inputs["W_q"]).T * SCALE)[:, perm]).astype(bf)
    wk = np.ascontiguousarray(np.asarray(inputs["W_k"]).T[:, perm]).astype(bf)
    wv = np.ascontiguousarray(np.asarray(inputs["W_v"]).T).astype(bf)
    wo = np.ascontiguousarray(np.asarray(inputs["W_out"]).T[perm, :]).astype(bf)
    in_maps = []
    for c in range(NCORES):
        xc = hs[BPC * c:BPC * (c + 1)].reshape(TOK, HS).T
        in_maps.append({
            "xT": np.ascontiguousarray(xc).astype(bf),
            "wq": wq, "wk": wk, "wv": wv, "wo": wo,
        })
    return in_maps


def run(inputs, **kw):
    nc = _build()
    in_maps = _prep_in_maps(inputs)
    res = run_bass_kernel_spmd(nc, in_maps, list(range(NCORES)), **kw)
    outs = [res.results[c]["out"].reshape(BPC, S, HS) for c in range(NCORES)]
    full = np.concatenate(outs, axis=0).astype(np.float32)
    full = full + np.asarray(inputs["b_out"], dtype=np.float32)[None, None, :]
    return full, res


def kernel(**inputs) -> np.ndarray:
    full, _ = run(inputs)
    return full



# revision 18
# speedup vs baseline: 1.2809x; 1.2809x over previous
"""Trainium2 Bass kernel for LoRAIPAttnProcessor (reduces to plain MHA).

Math (LORA_SCALE=0, IP_SCALE=0, b_out=0 contributions handled host-side):
  q = x @ Wq.T * scale ; k = x @ Wk.T ; v = x @ Wv.T
  P = softmax(q k^T) per head (8 heads, head_dim 160)
  out = (P v) @ Wout.T + b_out

Sharding: data-parallel over batch. 16 batches -> 8 cores x 2 batches.

Device layout strategy (zero on-device transposes):
  - host supplies xT [1280, 2048] (features on partitions) in bf16
  - host supplies Wq.T/Wk.T with *columns permuted* so each head's first 128
    output dims form full 128-partition tiles 0..7 and the 8x32 tails pack
    into tiles 8,9.  Wout.T gets the matching *row* permutation.
  - scores are computed transposed: ST[j,i] = k q^T  (keys on partitions), so
    softmax exp is a pure elementwise ACT op and P[j,i] feeds the PV matmul
    directly as the moving operand: OT[d,i] = v[j,d].T @ P[j,i].
  - a ones-column appended to v gives the softmax denominator as an extra
    output row of OT; normalization happens on DVE via a PE rank-1-broadcast
    reciprocal after otm/ott are evicted to SBUF.
  - out-projection consumes OT tiles as stationary -> final lands [token, ch].

Perf notes (HW-measured; the CoreSim cost model misses most of these):
  - matmuls whose contraction uses <128 partitions stream at ~half rate, so
    the 32-row k-tails are stored zero-padded to 128 partitions at their
    packed row offset (ktp tiles); the packed q-tail tile then works as the
    moving operand unchanged (garbage rows hit zero weights).
  - every matmul pays its own ldweights (no dedup, walrus ldw-opt off), so
    v-projection uses 512-wide psum chunks (30 matmuls/j-tile instead of 80).
  - pv(h, j) lags one scores step globally (across head boundaries) and
    otm/ott psums are evicted to SBUF immediately, so the PE never stalls on
    the last exp of a head while normalization runs off-path on DVE.
  - x/wq DMAs are issue-interleaved and wpool holds 20 tiles so weight
    streams (wk/wv/wo) prefetch behind compute.
"""

import numpy as np
import ml_dtypes
from contextlib import ExitStack

import concourse.bass as bass
import concourse.bacc as bacc
import concourse.mybir as mybir
import concourse.tile as tile
from concourse.bass_utils import run_bass_kernel_spmd

HS = 1280
HEADS = 8
D = HS // HEADS           # 160
B = 16
S = 1024
NCORES = 8
BPC = B // NCORES         # 2 batches per core
TOK = BPC * S             # 2048 tokens per core
SCALE = D ** -0.5
CT = HS // 128            # 10 feature tiles
IC = 512                  # i (query) chunk for psum
JT = S // 128             # 8 key tiles per batch
MT = S // 128             # 8 token tiles per batch

BF16 = mybir.dt.bfloat16
F32 = mybir.dt.float32
EXP = mybir.ActivationFunctionType.Exp

VW = D + 1                # 161: per-head v width incl ones column


def _perm():
    """Output-feature permutation: head mains to tiles 0..7, tails packed 8..9."""
    p = []
    for h in range(HEADS):
        p.extend(range(D * h, D * h + 128))
    for h in range(HEADS):
        p.extend(range(D * h + 128, D * h + D))
    return np.array(p, dtype=np.int64)


def _body(ctx, tc, xT_d, wq_d, wk_d, wv_d, wo_d, out_d):
    nc = tc.nc

    wpool = ctx.enter_context(tc.tile_pool(name="w", bufs=20))
    xpool = ctx.enter_context(tc.tile_pool(name="x", bufs=CT))
    qpool = ctx.enter_context(tc.tile_pool(name="q", bufs=CT))
    kpool = ctx.enter_context(tc.tile_pool(name="k", bufs=CT - 2))
    tpool = ctx.enter_context(tc.tile_pool(name="ktp", bufs=HEADS))
    vpool = ctx.enter_context(tc.tile_pool(name="v", bufs=JT))
    opool = ctx.enter_context(tc.tile_pool(name="ot", bufs=CT))
    ppool = ctx.enter_context(tc.tile_pool(name="p", bufs=3))
    rpool = ctx.enter_context(tc.tile_pool(name="recip", bufs=2))
    bpool = ctx.enter_context(tc.tile_pool(name="bcast", bufs=2))
    epool = ctx.enter_context(tc.tile_pool(name="evict", bufs=2))
    pr_ps = ctx.enter_context(tc.tile_pool(name="pr_ps", bufs=2, space="PSUM"))
    st_ps = ctx.enter_context(tc.tile_pool(name="st_ps", bufs=2, space="PSUM"))
    om_ps = ctx.enter_context(tc.tile_pool(name="om_ps", bufs=2, space="PSUM"))
    ot_ps = ctx.enter_context(tc.tile_pool(name="ot_ps", bufs=2, space="PSUM"))

    ones = rpool.tile([1, 128], F32, tag="ones", name="ones")
    nc.vector.memset(ones[:], 1.0)

    # k-tail tiles, zero-padded to full 128 contraction partitions: head h's
    # 32 tail rows live at their packed offset 32*(h%4); all other rows stay
    # zero so the packed q-tail tile works as the moving operand unchanged.
    # Allocated once; zeros persist across batches (only data rows rewritten).
    ktp = [tpool.tile([128, S], BF16, tag="ktp", name="ktp") for _ in range(HEADS)]
    for h in range(HEADS):
        nc.vector.memset(ktp[h][:], 0.0)

    for b in range(BPC):
        # ---- load this batch's xT, interleaved with wq so the first q-proj
        # accumulation chain can start as soon as tile pair 0 lands ----
        xb = []
        wq_t, wk_t, wv_t = [], [], []
        for c in range(CT):
            t = xpool.tile([128, S], BF16, tag="xb", name="xb")
            nc.sync.dma_start(out=t[:], in_=xT_d[c * 128:(c + 1) * 128, b * S:(b + 1) * S])
            xb.append(t)
            t = wpool.tile([128, HS], BF16, tag="w", name="w")
            nc.sync.dma_start(out=t[:], in_=wq_d[c * 128:(c + 1) * 128, :])
            wq_t.append(t)
        for c in range(CT):
            t = wpool.tile([128, HS], BF16, tag="w", name="w")
            nc.sync.dma_start(out=t[:], in_=wk_d[c * 128:(c + 1) * 128, :])
            wk_t.append(t)
        for c in range(CT):
            t = wpool.tile([128, HS], BF16, tag="w", name="w")
            nc.sync.dma_start(out=t[:], in_=wv_d[c * 128:(c + 1) * 128, :])
            wv_t.append(t)

        # ---- q/k projections: dst[m][dout 128, i] = W.T[c, dout_m] . xT[c, i] ----
        qT, kT = [], []
        for wt, dst, dtag in ((wq_t, qT, "qT"), (wk_t, kT, "kT")):
            for m in range(CT):
                is_k_tail = dst is kT and m >= 8
                if is_k_tail:
                    dtile = None
                else:
                    dtile = (qpool if dst is qT else kpool).tile(
                        [128, S], BF16, tag=dtag, name=dtag)
                dst.append(dtile)
                for ic in range(S // IC):
                    ps = pr_ps.tile([128, IC], F32, tag="pr", name="pr")
                    for c in range(CT):
                        nc.tensor.matmul(
                            ps[:],
                            wt[c][:, m * 128:(m + 1) * 128],
                            xb[c][:, ic * IC:(ic + 1) * IC],
                            start=(c == 0), stop=(c == CT - 1),
                        )
                    if is_k_tail:
                        # scatter the four heads' tail rows into their padded tiles
                        for hh in range(4):
                            h = (m - 8) * 4 + hh
                            r = 32 * hh
                            nc.vector.tensor_copy(
                                ktp[h][r:r + 32, ic * IC:(ic + 1) * IC],
                                ps[r:r + 32, :])
                    else:
                        nc.vector.tensor_copy(dtile[:, ic * IC:(ic + 1) * IC], ps[:])

        # ---- v projection: v'[j][tok 128, h*161 + d] (+ ones col per head)
        # 512-wide psum chunks (3.2 heads each); evictions split on head
        # boundaries so the strided 161-per-head layout lands right ----
        vp = []
        for j in range(JT):
            vt = vpool.tile([128, HEADS * VW], BF16, tag="vp", name="vp")
            vp.append(vt)
            for n0, nw in ((0, 512), (512, 512), (1024, 256)):
                ps = pr_ps.tile([128, nw], F32, tag="pr", name="pr")
                for c in range(CT):
                    nc.tensor.matmul(
                        ps[:],
                        xb[c][:, j * 128:(j + 1) * 128],
                        wv_t[c][:, n0:n0 + nw],
                        start=(c == 0), stop=(c == CT - 1),
                    )
                # scatter chunk cols [n0, n0+nw) into per-head 161-strided vt
                col = n0
                while col < n0 + nw:
                    h, d0 = divmod(col, D)
                    seg = min(D - d0, n0 + nw - col)
                    nc.vector.tensor_copy(
                        vt[:, h * VW + d0:h * VW + d0 + seg],
                        ps[:, col - n0:col - n0 + seg])
                    col += seg
            for h in range(HEADS):
                nc.vector.memset(vt[:, h * VW + D:(h + 1) * VW], 1.0)

        # ---- attention, software-pipelined across heads ----
        # pv(h, j) lags one scores step behind globally, so head h's final pv
        # is emitted after head h+1's first scores block and the PE stream
        # never waits on the exp of the last j.  otm/ott psums are evicted to
        # SBUF immediately after the final pv so the next head's accumulation
        # can claim the banks while normalization runs on DVE off-path.
        OT = [opool.tile([128, S], BF16, tag="ot", name="ot") for _ in range(CT)]
        hstate = {}

        def emit_pv(h, j):
            stt = hstate[h]
            for ic in range(2):
                nc.tensor.matmul(
                    stt["otm"][ic][:],
                    vp[j][:, h * VW:h * VW + 128],
                    stt["pj"][j][:, ic * IC:(ic + 1) * IC],
                    start=(j == 0), stop=(j == JT - 1),
                )
                nc.tensor.matmul(
                    stt["ott"][ic][:],
                    vp[j][:, h * VW + 128:(h + 1) * VW],
                    stt["pj"][j][:, ic * IC:(ic + 1) * IC],
                    start=(j == 0), stop=(j == JT - 1),
                )

        def finish_head(h):
            stt = hstate.pop(h)
            g = 8 + h // 4          # tail tile index
            r = 32 * (h % 4)        # tail row offset
            for ic in range(2):
                ou = bpool.tile([128, IC], F32, tag="ou", name="ou")
                ol = rpool.tile([33, IC], F32, tag="ol", name="ol")
                nc.vector.tensor_copy(ou[:], stt["otm"][ic][:])
                nc.vector.tensor_copy(ol[:], stt["ott"][ic][:])
                rc = rpool.tile([1, IC], F32, tag="rc", name="rc")
                nc.vector.reciprocal(rc[:], ol[32:33, :])
                # rank-1 broadcast on PE: ones.T @ rc -> [128, IC] psum
                bc_ps = pr_ps.tile([128, IC], F32, tag="pr", name="pr")
                nc.tensor.matmul(bc_ps[:], ones[:], rc[:], start=True, stop=True)
                sl = slice(ic * IC, (ic + 1) * IC)
                nc.vector.tensor_mul(OT[h][:, sl], ou[:], bc_ps[:])
                nc.vector.tensor_mul(OT[g][r:r + 32, sl], ol[0:32, :], bc_ps[0:32, :])

        pend = None
        for h in range(HEADS):
            km, kt = kT[h], ktp[h]
            qm, qt = qT[h], qT[8 + h // 4]
            for j in range(JT):
                pj_t = ppool.tile([128, S], BF16, tag="pj", name="pj")
                for ic in range(2):
                    st = st_ps.tile([128, IC], F32, tag="st", name="st")
                    nc.tensor.matmul(
                        st[:],
                        km[:, j * 128:(j + 1) * 128],
                        qm[:, ic * IC:(ic + 1) * IC],
                        start=True, stop=False,
                    )
                    # tail contraction at full 128 partitions: kt rows outside
                    # this head's 32 tail rows are zero, so qt's other heads'
                    # rows contribute nothing.
                    nc.tensor.matmul(
                        st[:],
                        kt[:, j * 128:(j + 1) * 128],
                        qt[:, ic * IC:(ic + 1) * IC],
                        start=False, stop=True,
                    )
                    nc.scalar.activation(pj_t[:, ic * IC:(ic + 1) * IC], st[:], EXP)
                if pend is not None:
                    emit_pv(*pend)
                    if pend[1] == JT - 1:
                        finish_head(pend[0])
                if j == 0:
                    # allocate after the previous head's final pv + eviction
                    # are emitted, so the psum slot handoff is ordered
                    hstate[h] = {
                        "otm": [om_ps.tile([128, IC], F32, tag="om", name="om")
                                for _ in range(2)],
                        "ott": [ot_ps.tile([33, IC], F32, tag="otl", name="otl")
                                for _ in range(2)],
                        "pj": [None] * JT,
                    }
                hstate[h]["pj"][j] = pj_t
                pend = (h, j)
        emit_pv(*pend)
        finish_head(pend[0])

        # ---- out projection: out[i, cout] = OT[d, i].T . Wout.T[d, cout] ----
        wo_t = []
        for c in range(CT):
            t = wpool.tile([128, HS], BF16, tag="w", name="w")
            nc.sync.dma_start(out=t[:], in_=wo_d[c * 128:(c + 1) * 128, :])
            wo_t.append(t)
        for it in range(MT):
            for n0, nw in ((0, 512), (512, 512), (1024, 256)):
                ps = pr_ps.tile([128, nw], F32, tag="pr", name="pr")
                for c in range(CT):
                    nc.tensor.matmul(
                        ps[:],
                        OT[c][:, it * 128:(it + 1) * 128],
                        wo_t[c][:, n0:n0 + nw],
                        start=(c == 0), stop=(c == CT - 1),
                    )
                ev = epool.tile([128, nw], F32, tag="ev", name="ev")
                nc.vector.tensor_copy(ev[:], ps[:])
                nc.sync.dma_start(
                    out=out_d[b * S + it * 128: b * S + (it + 1) * 128, n0:n0 + nw],
                    in_=ev[:],
                )


_CACHE = {}


def _build(reps=1):
    key = ("nc", reps)
    if key in _CACHE:
        return _CACHE[key]
    nc = bacc.Bacc(None)
    xT_d = nc.declare_dram_parameter("xT", [HS, TOK], BF16, isOutput=False)
    wq_d = nc.declare_dram_parameter("wq", [HS, HS], BF16, isOutput=False)
    wk_d = nc.declare_dram_parameter("wk", [HS, HS], BF16, isOutput=False)
    wv_d = nc.declare_dram_parameter("wv", [HS, HS], BF16, isOutput=False)
    wo_d = nc.declare_dram_parameter("wo", [HS, HS], BF16, isOutput=False)
    out_d = nc.declare_dram_parameter("out", [TOK, HS], F32, isOutput=True)
    with tile.TileContext(nc) as tc:
        for _ in range(reps):
            with ExitStack() as ctx:
                _body(ctx, tc, xT_d[:], wq_d[:], wk_d[:], wv_d[:], wo_d[:], out_d[:])
    nc.compile()
    _CACHE[key] = nc
    return nc


def _prep_in_maps(inputs):
    hs = np.asarray(inputs["hidden_states"], dtype=np.float32)
    perm = _perm()
    bf = ml_dtypes.bfloat16
    wq = np.ascontiguousarray((np.asarray(inputs["W_q"]).T * SCALE)[:, perm]).astype(bf)
    wk = np.ascontiguousarray(np.asarray(inputs["W_k"]).T[:, perm]).astype(bf)
    wv = np.ascontiguousarray(np.asarray(inputs["W_v"]).T).astype(bf)
    wo = np.ascontiguousarray(np.asarray(inputs["W_out"]).T[perm, :]).astype(bf)
    in_maps = []
    for c in range(NCORES):
        xc = hs[BPC * c:BPC * (c + 1)].reshape(TOK, HS).T
        in_maps.append({
            "xT": np.ascontiguousarray(xc).astype(bf),
            "wq": wq, "wk": wk, "wv": wv, "wo": wo,
        })
    return in_maps


def run(inputs, **kw):
    nc = _build()
    in_maps = _prep_in_maps(inputs)
    res = run_bass_kernel_spmd(nc, in_maps, list(range(NCORES)), **kw)
    outs = [res.results[c]["out"].reshape(BPC, S, HS) for c in range(NCORES)]
    full = np.concatenate(outs, axis=0).astype(np.float32)
    full = full + np.asarray(inputs["b_out"], dtype=np.float32)[None, None, :]
    return full, res


def kernel(**inputs) -> np.ndarray:
    full, _ = run(inputs)
    return full

